# revision 1
# baseline (speedup 1.0000x reference)
"""CrossVariableAttention Trainium2 kernel (Bass/Tile), 8-core data parallel.

Reference computation (per batch b, with L = d_model = 512, N = 2048):
    xt   = x[b].T                            # [N, L]
    qkv  = xt @ W_qkv + b_qkv                # [N, 3L]
    q, k, v = split(qkv)
    attn = softmax(q @ k.T / sqrt(L))        # [N, N]
    out  = (attn @ v) @ W_proj + b_proj      # [N, L]
    y[b] = out.T                             # [L, N]

Mapping: batch B=8 -> one batch element per NeuronCore (SPMD, same program,
different inputs).  All on-chip matmuls run in float32r (full-rate fp32 on
the PE at free-dim >= 256, ~1e-4 relative error).

Per-core dataflow, everything in "transposed land" so no PE transposes are
needed:
    X    = x[b]               [d_in=512, n=2048]   (native input layout)
    QT   = Wq'^T.X + bq'      [d, n]   (Wq', bq' pre-scaled by 1/sqrt(512))
    KT   = Wk^T.X + bk        [d, n]
    V    = X^T.Wv + bv        [m, d]   (native layout, X used as lhsT)
    per slab of 512 queries:
      S^T  = KT^T.QT          [m, n_slab]  (16 psum tiles [128, 512])
      Pt   = exp(S^T)         (softmax without max-subtraction: logits O(1))
      den  = ones^T.Pt        [1, n_slab]  (column sums via PE)
      AO^T = V^T.Pt           [d, n_slab]
      Y^T  = Wproj^T.AO^T     [dout, n_slab]
      y    = Y^T * (1/den) + bproj  -> DMA out (already in [L, N] layout)
"""

from contextlib import ExitStack

import numpy as np

import concourse.bass as bass
import concourse.mybir as mybir
import concourse.tile as tile
from concourse.bass import ds
from concourse.bass_utils import run_bass_kernel_spmd
from concourse.vector_clock import ScopedClock

F32 = mybir.dt.float32
F32R = mybir.dt.float32r
AF = mybir.ActivationFunctionType

B = 8
P = 128
D = 512
N = 2048
DCH = D // P
NCH = N // P
NSLAB = N // 512

# ---------------------------------------------------------------------------
# The walrus build in this environment accepts at most ONE sync wait per
# instruction (setupSyncWait: "Too many sync wait commands").  Tile attaches
# several.  Fix: split excess waits onto engine-local NOPs placed just before
# the instruction (same engine => same stream order => identical semantics).
MAX_WAITS_PER_INST = 1


class SplitDrainTileContext(tile.TileContext):
    def _drain_and_barrier(self, tick_clock, wait_clock):
        nc = self.nc
        probe = nc.sync.nop(nofuse=True, hint="split_drain_waits")
        wait_clock.add_sem_waits(
            probe.ins, ScopedClock({None: tick_clock.global_clock})
        )
        waits = list(probe.ins.sync_info.on_wait)
        probe.ins.sync_info.on_wait = waits[:MAX_WAITS_PER_INST]
        for i in range(MAX_WAITS_PER_INST, len(waits), MAX_WAITS_PER_INST):
            extra = nc.sync.nop(nofuse=True, hint="split_drain_waits")
            extra.ins.sync_info = mybir.SyncInfo(
                on_wait=waits[i : i + MAX_WAITS_PER_INST], on_update=[]
            )
        nc.sync.drain()
        nc.all_engine_barrier()
        assert self.sems is not None
        popped = nc._tile_sem_poison_stack.pop()
        assert popped is self._sem_poison
        nc.clear_and_free_semaphores(list(self.sems.allocated().values()))
        nc.all_engine_barrier()


def split_sync_waits(nc, max_waits=MAX_WAITS_PER_INST):
    for fn in nc.m.functions:
        for bb in fn.blocks:
            insts = list(bb.instructions)
            out = []
            changed = False
            for inst in insts:
                si = getattr(inst, "sync_info", None)
                if si is not None:
                    waits = list(si.on_wait or [])
                    if len(waits) > max_waits:
                        changed = True
                        for j, w in enumerate(waits[: len(waits) - max_waits]):
                            out.append(
                                mybir.InstNoOp(
                                    name=f"{inst.name}-sw{j}",
                                    engine=inst.engine,
                                    bass_nofuse=True,
                                    sync_info=mybir.SyncInfo(
                                        on_wait=[w], on_update=[]
                                    ),
                                )
                            )
                        si.on_wait = waits[len(waits) - max_waits :]
                out.append(inst)
            if changed:
                bb.instructions = out


def build_nc():
    nc = bass.Bass()

    x = nc.declare_dram_parameter("x", [D, N], F32R, isOutput=False)
    wq = nc.declare_dram_parameter("wq", [D, D], F32R, isOutput=False)
    wk = nc.declare_dram_parameter("wk", [D, D], F32R, isOutput=False)
    wv = nc.declare_dram_parameter("wv", [D, D], F32R, isOutput=False)
    wp = nc.declare_dram_parameter("wp", [D, D], F32R, isOutput=False)
    bq = nc.declare_dram_parameter("bq", [D], F32, isOutput=False)
    bk = nc.declare_dram_parameter("bk", [D], F32, isOutput=False)
    bv = nc.declare_dram_parameter("bv", [D], F32, isOutput=False)
    bp = nc.declare_dram_parameter("bp", [D], F32, isOutput=False)
    ones_in = nc.declare_dram_parameter("ones", [P, 1], F32R, isOutput=False)
    y = nc.declare_dram_parameter("y", [D, N], F32, isOutput=True)
    recip_dram = nc.dram_tensor("recip_scratch", [NSLAB, 512], F32)

    with SplitDrainTileContext(nc) as tc, ExitStack() as ctx:
        consts = ctx.enter_context(tc.tile_pool(name="consts", bufs=1))
        qkv_sb = ctx.enter_context(tc.tile_pool(name="qkv", bufs=1))
        small = ctx.enter_context(tc.tile_pool(name="small", bufs=2))

        wv_sb = consts.tile([P, DCH, D], F32R, tag="wv")
        wp_sb = consts.tile([P, DCH, D], F32R, tag="wp")
        nc.sync.dma_start(out=wv_sb, in_=wv.rearrange("(c p) o -> p c o", p=P))
        nc.sync.dma_start(out=wp_sb, in_=wp.rearrange("(c p) o -> p c o", p=P))

        bq_sb = consts.tile([P, DCH], F32, tag="bq")
        bk_sb = consts.tile([P, DCH], F32, tag="bk")
        bp_sb = consts.tile([P, DCH], F32, tag="bp")
        nc.sync.dma_start(out=bq_sb, in_=bq.rearrange("(c p) -> p c", p=P))
        nc.sync.dma_start(out=bk_sb, in_=bk.rearrange("(c p) -> p c", p=P))
        nc.sync.dma_start(out=bp_sb, in_=bp.rearrange("(c p) -> p c", p=P))

        bv_bc = consts.tile([P, D], F32, tag="bv")
        bv_ap = bv[:]
        bv_bcast = bass.AP(
            tensor=bv_ap.tensor, offset=bv_ap.offset, ap=[[0, P], bv_ap.ap[0]]
        )
        nc.sync.dma_start(out=bv_bc, in_=bv_bcast)

        ones = consts.tile([P, 1], F32R, tag="ones")
        nc.sync.dma_start(out=ones, in_=ones_in[:, :])

        # --- phase 1: QT, KT (transposed layout), V (natural layout) -----
        qt_sb = qkv_sb.tile([P, DCH, N], F32R, tag="qt")
        kt_sb = qkv_sb.tile([P, DCH, N], F32R, tag="kt")
        v_sb = qkv_sb.tile([P, NCH, D], F32R, tag="v")

        with tc.tile_pool(name="xin", bufs=1) as xin, \
             tc.tile_pool(name="ps1", bufs=4, space="PSUM") as ps1:
            x_sb = xin.tile([P, DCH, N], F32R, tag="x")
            nc.sync.dma_start(out=x_sb, in_=x.rearrange("(c p) n -> p c n", p=P))
            wq_sb = xin.tile([P, DCH, D], F32R, tag="wq")
            wk_sb = xin.tile([P, DCH, D], F32R, tag="wk")
            nc.sync.dma_start(out=wq_sb, in_=wq.rearrange("(c p) o -> p c o", p=P))
            nc.sync.dma_start(out=wk_sb, in_=wk.rearrange("(c p) o -> p c o", p=P))

            for w_sb, b_sb, o_sb in ((wq_sb, bq_sb, qt_sb), (wk_sb, bk_sb, kt_sb)):
                for oc in range(DCH):
                    for nb in range(NSLAB):
                        ps = ps1.tile([P, 512], F32, tag="ps1")
                        for ic in range(DCH):
                            nc.tensor.matmul(
                                ps,
                                w_sb[:, ic, ds(oc * P, P)],
                                x_sb[:, ic, ds(nb * 512, 512)],
                                start=(ic == 0),
                                stop=(ic == DCH - 1),
                            )
                        nc.scalar.activation(
                            out=o_sb[:, oc, ds(nb * 512, 512)],
                            in_=ps,
                            func=AF.Identity,
                            bias=b_sb[:, oc : oc + 1],
                            scale=1.0,
                        )

            for mc in range(NCH):
                ps = ps1.tile([P, 512], F32, tag="ps1")
                for ic in range(DCH):
                    nc.tensor.matmul(
                        ps,
                        x_sb[:, ic, ds(mc * P, P)],
                        wv_sb[:, ic, :],
                        start=(ic == 0),
                        stop=(ic == DCH - 1),
                    )
                nc.vector.tensor_add(out=v_sb[:, mc, :], in0=ps, in1=bv_bc)

        # --- phase 2: attention, per slab of 512 queries ------------------
        with tc.tile_pool(name="pt", bufs=18) as pt_pool, \
             tc.tile_pool(name="ao", bufs=1) as ao_pool, \
             tc.tile_pool(name="outp", bufs=2) as outp, \
             tc.tile_pool(name="ps_st", bufs=3, space="PSUM") as ps_st, \
             tc.tile_pool(name="ps_den", bufs=1, space="PSUM") as ps_den, \
             tc.tile_pool(name="ps_pv", bufs=2, space="PSUM") as ps_pv, \
             tc.tile_pool(name="ps_y", bufs=2, space="PSUM") as ps_y:
            for nb in range(NSLAB):
                nsl = ds(nb * 512, 512)

                pt_tiles = []
                for mc in range(NCH):
                    ps = ps_st.tile([P, 512], F32, tag="st")
                    for dc in range(DCH):
                        nc.tensor.matmul(
                            ps,
                            kt_sb[:, dc, ds(mc * P, P)],
                            qt_sb[:, dc, nsl],
                            start=(dc == 0),
                            stop=(dc == DCH - 1),
                        )
                    pt = pt_pool.tile([P, 512], F32R, tag="pt")
                    nc.scalar.activation(out=pt, in_=ps, func=AF.Exp)
                    pt_tiles.append(pt)

                ps_d = ps_den.tile([1, 512], F32, tag="den")
                for mc in range(NCH):
                    nc.tensor.matmul(
                        ps_d,
                        ones[:, :],
                        pt_tiles[mc][:, :],
                        start=(mc == 0),
                        stop=(mc == NCH - 1),
                    )
                recip = small.tile([1, 512], F32, tag="recip")
                nc.vector.reciprocal(out=recip, in_=ps_d)
                nc.sync.dma_start(out=recip_dram[nb], in_=recip)
                recip_bc = small.tile([P, 512], F32, tag="recip_bc")
                rd = recip_dram[nb]
                rd_bcast = bass.AP(
                    tensor=rd.tensor, offset=rd.offset, ap=[[0, P], rd.ap[-1]]
                )
                nc.sync.dma_start(out=recip_bc, in_=rd_bcast)

                ao = ao_pool.tile([P, DCH, 512], F32R, tag="ao")
                for dc in range(DCH):
                    ps = ps_pv.tile([P, 512], F32, tag="pv")
                    for mc in range(NCH):
                        nc.tensor.matmul(
                            ps,
                            v_sb[:, mc, ds(dc * P, P)],
                            pt_tiles[mc][:, :],
                            start=(mc == 0),
                            stop=(mc == NCH - 1),
                        )
                    nc.scalar.copy(out=ao[:, dc, :], in_=ps)

                for oc in range(DCH):
                    ps = ps_y.tile([P, 512], F32, tag="y")
                    for dc in range(DCH):
                        nc.tensor.matmul(
                            ps,
                            wp_sb[:, dc, ds(oc * P, P)],
                            ao[:, dc, :],
                            start=(dc == 0),
                            stop=(dc == DCH - 1),
                        )
                    t = outp.tile([P, 512], F32, tag="out")
                    nc.vector.tensor_tensor(
                        out=t, in0=ps, in1=recip_bc, op=mybir.AluOpType.mult
                    )
                    nc.vector.tensor_scalar_add(
                        out=t, in0=t, scalar1=bp_sb[:, oc : oc + 1]
                    )
                    nc.sync.dma_start(out=y[ds(oc * P, P), nsl], in_=t)

    split_sync_waits(nc)
    return nc


_NC_CACHE = None


def _get_nc():
    global _NC_CACHE
    if _NC_CACHE is None:
        _NC_CACHE = build_nc()
    return _NC_CACHE


def kernel(**inputs):
    x = np.ascontiguousarray(np.asarray(inputs["x"], dtype=np.float32))
    W_qkv = np.asarray(inputs["W_qkv"], dtype=np.float32)
    b_qkv = np.asarray(inputs["b_qkv"], dtype=np.float32)
    W_proj = np.asarray(inputs["W_proj"], dtype=np.float32)
    b_proj = np.asarray(inputs["b_proj"], dtype=np.float32)

    s = np.float32(1.0) / np.sqrt(np.float32(D))
    shared = {
        "wq": np.ascontiguousarray(W_qkv[:, :D] * s),
        "wk": np.ascontiguousarray(W_qkv[:, D : 2 * D]),
        "wv": np.ascontiguousarray(W_qkv[:, 2 * D :]),
        "wp": np.ascontiguousarray(W_proj),
        "bq": np.ascontiguousarray(b_qkv[:D] * s),
        "bk": np.ascontiguousarray(b_qkv[D : 2 * D]),
        "bv": np.ascontiguousarray(b_qkv[2 * D :]),
        "bp": np.ascontiguousarray(b_proj),
        "ones": np.ones((P, 1), np.float32),
    }
    in_maps = [{"x": x[b], **shared} for b in range(B)]

    nc = _get_nc()
    res = run_bass_kernel_spmd(nc, in_maps, core_ids=list(range(B)))
    return np.stack([res.results[b]["y"] for b in range(B)]).astype(np.float32)
